# revision 5
# baseline (speedup 1.0000x reference)
"""Trainium2 Bass kernel for GCN message passing (COO SpMM segment-sum).

out[i] = sum_{e: rows[e]==i} vals[e] * embeds[cols[e]]
N=100000 nodes, E=1600000 edges, D=64 features, f32.

Strategy (8 NeuronCores, SPMD):
  - Shard OUTPUT rows across cores: core k owns rows [12500k, 12500(k+1)).
    rows is sorted, so each core's edges are one contiguous slice.
  - Per core, rows split into W=98 windows of 128 output rows. The
    embeds table is split into CH=4 chunks of 25000 rows so gather
    indices fit in int16 (dma_gather requirement).
  - The embeds table is pre-packed on the host as overlapping bf16
    pairs: table[i, 0:64] = bf16(embeds[i]), table[i, 64:128] =
    bf16(embeds[i+1]). Rows are 256B (the dma_gather minimum), and a
    gather of row c directly yields bf16(embeds[c]) in the first 64
    columns -- bf16 compute with no extra traffic vs f32 rows.
  - Host packs each (window, chunk)'s edges into T_c tiles of 128 slots
    (slot i of a tile run: partition i%128, tile i//128), padded with
    val=0 slots so every window has the same T_c tiles per chunk.
    Global tile id: gt = C_off[c] + w*T_c[c] + j  (chunk-major).
  - On core, per span of GW windows: one dma_gather per chunk fetches
    all of the span's embeds rows into SBUF. Gather calls round-robin
    over 4 SWDGE queues: a single queue drains ~32 GB/s (descriptor-
    serial); 4 queues reach ~126 GB/s. Per window, per 128-edge tile:
    build M[p,j] = (j == rloc[p]) * vals[p] (bf16) with one fused
    tensor_scalar against an iota row, then matmul-accumulate
    psum[j,d] += M.T @ G[:, :64] over the window's T tiles.
  - psum copied into a [128, W*64] SBUF accumulator; one final DMA out.
    Host re-interleaves [128, W, 64] -> [W*128, 64] and concatenates.
"""

import os

import numpy as np

N_NODES = 100000
N_EDGES = 1600000
D = 64
P = 128
NC = 8
RPC = N_NODES // NC  # rows per core = 12500
W = -(-RPC // P)  # windows per core = 98
CH = 4
CHROWS = N_NODES // CH  # 25000 rows per gather chunk (< 32768 for int16)
NQ = 4  # SWDGE queues (ucode max); per-queue gather drain is ~32 GB/s

# Stash of the last run's BassKernelResults for test.py.
LAST_RESULTS = None


def build_program(ch_rows, w, t_cs, gw, d=D, reps=1, bufs_g=3, bufs_m=16, bufs_ps=8, nq=NQ, skip_gather=False, skip_compute=False, sp_split=False):
    """Build the single-core SPMD Bass program (same on all 8 cores).

    ch_rows: rows per embeds chunk; t_cs: tiles per window per chunk.
    reps > 1 wraps the whole body in a hardware loop (benchmarking only).
    """
    import concourse.bacc as bacc
    import concourse.mybir as mybir
    import concourse.tile as tile

    f32 = mybir.dt.float32
    bf16 = mybir.dt.bfloat16
    i16 = mybir.dt.int16
    i32 = mybir.dt.int32

    ch = len(t_cs)
    t_tot = sum(t_cs)
    c_off = np.concatenate([[0], np.cumsum([w * t for t in t_cs])])

    nc = bacc.Bacc(num_swdge_queues=nq)
    emb_ds = [
        nc.declare_dram_parameter(f"emb{c}", [ch_rows, 2 * d], bf16, isOutput=False)
        for c in range(ch)
    ]
    n_slots = w * t_tot * P
    idx_d = nc.declare_dram_parameter("idx", [P, n_slots // 16], i16, isOutput=False)
    # meta[:, :w*t_tot] = rloc, meta[:, w*t_tot:] = vals — one DMA, one sem.
    meta_d = nc.declare_dram_parameter("meta", [P, 2 * w * t_tot], f32, isOutput=False)
    out_d = nc.declare_dram_parameter("out", [P, w * d], f32, isOutput=True)

    assert w % gw == 0
    qctr = [0]
    with tile.TileContext(nc) as tc:
        with (
            tc.tile_pool(name="const", bufs=1) as cpool,
            tc.tile_pool(name="gath", bufs=bufs_g) as gpool,
            tc.tile_pool(name="m", bufs=bufs_m) as mpool,
            tc.tile_pool(name="ps", bufs=bufs_ps, space="PSUM") as ppool,
        ):
            idx_sb = cpool.tile([P, n_slots // 16], i16, name="idx_sb")
            nc.sync.dma_start(out=idx_sb[:], in_=idx_d[:])
            meta_sb = cpool.tile([P, 2 * w * t_tot], f32, name="meta_sb")
            nc.sync.dma_start(out=meta_sb[:], in_=meta_d[:])
            rloc_sb = meta_sb[:, : w * t_tot]
            vals_sb = meta_sb[:, w * t_tot :]
            # iota row 0..127 on every partition, built on-chip (bf16 exact).
            iota32 = cpool.tile([P, P], i32, name="iota32")
            nc.gpsimd.iota(iota32[:], pattern=[[1, P]], base=0, channel_multiplier=0)
            iota_sb = cpool.tile([P, P], bf16, name="iota_sb")
            nc.gpsimd.tensor_copy(out=iota_sb[:], in_=iota32[:])
            out_sb = cpool.tile([P, w * d], f32, name="out_sb")
            if skip_compute:
                nc.gpsimd.memset(out_sb[:], 0.0)

            def body():
                for s in range(w // gw):
                    gb3 = []
                    for c in range(ch):
                        gb = gpool.tile(
                            [P, gw * t_cs[c] * 2 * d], bf16, name=f"gb{c}", tag=f"gb{c}"
                        )
                        view = gb[:].rearrange("p (n x) -> p n x", x=2 * d)
                        gb3.append(view)
                        n_idx = gw * t_cs[c] * P
                        slot_base = (c_off[c] + s * gw * t_cs[c]) * P
                        if skip_gather:
                            continue
                        if sp_split:
                            # single_packet=True caps at 1024 indices (ring
                            # limit); split into 8-tile calls.
                            for t0 in range(0, gw * t_cs[c], 8):
                                tn = min(8, gw * t_cs[c] - t0)
                                ni = tn * P
                                sb2 = slot_base + t0 * P
                                nc.gpsimd.dma_gather(
                                    out_ap=view[:, t0 : t0 + tn, :],
                                    in_ap=emb_ds[c][:, :],
                                    idxs_ap=idx_sb[:, sb2 // 16 : (sb2 + ni) // 16],
                                    num_idxs=ni,
                                    num_idxs_reg=ni,
                                    elem_size=2 * d,
                                    single_packet=True,
                                    queue_num=qctr[0] % nq,
                                )
                                qctr[0] += 1
                        else:
                            nc.gpsimd.dma_gather(
                                out_ap=view,
                                in_ap=emb_ds[c][:, :],
                                idxs_ap=idx_sb[
                                    :, slot_base // 16 : (slot_base + n_idx) // 16
                                ],
                                num_idxs=n_idx,
                                num_idxs_reg=n_idx,
                                elem_size=2 * d,
                                single_packet=False,
                                queue_num=qctr[0] % nq,
                            )
                            qctr[0] += 1
                    for wi in range(gw):
                        if skip_compute:
                            break
                        wid = s * gw + wi
                        ps = ppool.tile([P, d], f32, space="PSUM", name="ps")
                        t_ctr = 0
                        for c in range(ch):
                            for j in range(t_cs[c]):
                                gt = int(c_off[c] + wid * t_cs[c] + j)
                                m = mpool.tile([P, P], bf16, name="m")
                                # Keep M-builds off Pool: Pool must stay free
                                # to issue gather descriptor-gens, or the
                                # gather pipeline stalls behind compute.
                                nc.vector.tensor_scalar(
                                    out=m[:],
                                    in0=iota_sb[:],
                                    scalar1=rloc_sb[:, gt : gt + 1],
                                    scalar2=vals_sb[:, gt : gt + 1],
                                    op0=mybir.AluOpType.is_equal,
                                    op1=mybir.AluOpType.mult,
                                )
                                nc.tensor.matmul(
                                    out=ps[:],
                                    lhsT=m[:],
                                    rhs=gb3[c][:, wi * t_cs[c] + j, 0:d],
                                    start=(t_ctr == 0),
                                    stop=(t_ctr == t_tot - 1),
                                )
                                t_ctr += 1
                        nc.scalar.copy(
                            out=out_sb[:, wid * d : (wid + 1) * d], in_=ps[:]
                        )
                nc.sync.dma_start(out=out_d[:], in_=out_sb[:])

            if reps == 1:
                body()
            else:
                with tc.For_i(0, reps, 1):
                    body()
    nc.compile()
    return nc


def prep_shards(rows, cols, vals):
    """Pack edges into chunk-major slot arrays.

    Returns (idx16 [NC,128,nslots/16], rloc [NC,128,W*T], vals [NC,128,W*T],
    t_cs) with slot (gt, p): gt = C_off[c] + w*T_c[c] + j.
    """
    rows = np.asarray(rows).astype(np.int64)
    cols = np.asarray(cols).astype(np.int64)
    vals = np.asarray(vals).astype(np.float32)
    e = rows.shape[0]

    k = rows // RPC
    lr = rows - k * RPC
    wv = lr // P
    rloc_v = lr - wv * P
    cv = cols // CHROWS
    idxloc = (cols - cv * CHROWS).astype(np.int16)

    # group edges by (k, c, w), ascending col within each run (HBM locality)
    perm = np.lexsort((idxloc, wv, cv, k))
    k_s, c_s, w_s = k[perm], cv[perm], wv[perm]
    key = (k_s * CH + c_s) * W + w_s
    counts = np.bincount(key, minlength=NC * CH * W)
    t_need = -(-counts // P).reshape(NC, CH, W)
    t_cs = [int(t_need[:, c, :].max()) for c in range(CH)]
    t_tot = sum(t_cs)
    c_off = np.concatenate([[0], np.cumsum([W * t for t in t_cs])])

    starts = np.concatenate([[0], np.cumsum(counts)])
    q = np.arange(e) - np.repeat(starts[:-1], counts)  # pos within (k,c,w) run
    j = q // P
    p = q % P
    tc_arr = np.array(t_cs)[c_s]
    gt = c_off[c_s] + w_s * tc_arr + j  # global tile id per edge

    n_slots = W * t_tot * P
    idx16 = np.zeros((NC, 16, n_slots // 16), np.int16)
    rloc = np.zeros((NC, P, W * t_tot), np.float32)
    v32 = np.zeros((NC, P, W * t_tot), np.float32)

    slot = gt * P + p  # global flat slot
    idx16[k_s, slot % 16, slot // 16] = idxloc[perm]
    rloc[k_s, p, gt] = rloc_v[perm].astype(np.float32)
    v32[k_s, p, gt] = vals[perm]
    # replicate the 16-partition index block for the 8 Q7 cores
    idx128 = np.tile(idx16, (1, 8, 1))
    return idx128, rloc, v32, t_cs


def make_emb_chunks(embeds):
    """Overlapping bf16 pair table: row i = [bf16(emb[i]) | bf16(emb[i+1])],
    split into CH contiguous chunks of CHROWS rows (256B each)."""
    import ml_dtypes

    emb = np.asarray(embeds).astype(ml_dtypes.bfloat16)
    tab = np.zeros((N_NODES, 2 * D), ml_dtypes.bfloat16)
    tab[:, :D] = emb
    tab[:-1, D:] = emb[1:]
    return [
        np.ascontiguousarray(tab[c * CHROWS : (c + 1) * CHROWS]) for c in range(CH)
    ]


def make_in_maps(rows, cols, vals, embeds):
    idx128, rloc, v32, t_cs = prep_shards(rows, cols, vals)
    emb_chunks = make_emb_chunks(embeds)
    in_maps = []
    for c in range(NC):
        m = {f"emb{i}": emb_chunks[i] for i in range(CH)}
        m["idx"] = np.ascontiguousarray(idx128[c])
        m["meta"] = np.ascontiguousarray(np.concatenate([rloc[c], v32[c]], axis=1))
        in_maps.append(m)
    return in_maps, t_cs


def unshard_output(results):
    blocks = []
    for c in range(NC):
        o = results[c]["out"].reshape(P, W, D)
        blocks.append(o.transpose(1, 0, 2).reshape(W * P, D)[:RPC])
    return np.ascontiguousarray(np.concatenate(blocks, axis=0), dtype=np.float32)


def kernel(rows, cols, vals, embeds):
    global LAST_RESULTS
    from concourse.bass_utils import run_bass_kernel_spmd

    in_maps, t_cs = make_in_maps(rows, cols, vals, embeds)
    gw = 7 if W % 7 == 0 else 1
    nc = build_program(CHROWS, W, t_cs, gw)

    res = run_bass_kernel_spmd(
        nc,
        in_maps,
        core_ids=list(range(NC)),
        trace=bool(int(os.environ.get("GCN_TRACE", "0"))),
    )
    LAST_RESULTS = res
    return unshard_output(res.results)


# revision 7
# speedup vs baseline: 1.9768x; 1.9768x over previous
"""Trainium2 Bass kernel for GCN message passing (COO SpMM segment-sum).

out[i] = sum_{e: rows[e]==i} vals[e] * embeds[cols[e]]
N=100000 nodes, E=1600000 edges, D=64 features, f32.

Strategy (8 NeuronCores, SPMD):
  - Shard OUTPUT rows across cores: core k owns rows [12500k, 12500(k+1)).
    rows is sorted, so each core's edges are one contiguous slice.
  - Per core, rows split into W=98 windows of 128 output rows. The
    embeds table is split into CH=4 chunks of 25000 rows so gather
    indices fit in int16 (dma_gather requirement).
  - The embeds table is pre-packed on the host as overlapping bf16
    pairs: table[i, 0:64] = bf16(embeds[i]), table[i, 64:128] =
    bf16(embeds[i+1]). Rows are 256B (the dma_gather minimum), and a
    gather of row c directly yields bf16(embeds[c]) in the first 64
    columns -- bf16 compute with no extra traffic vs f32 rows.
  - Host packs each (window, chunk)'s edges into T_c tiles of 128 slots
    (slot i of a tile run: partition i%128, tile i//128), padded with
    val=0 slots so every window has the same T_c tiles per chunk.
  - On core, per span of GW windows: one dma_gather per chunk fetches
    all of the span's embeds rows into SBUF. Gather calls round-robin
    over 4 SWDGE queues: a single queue drains ~32 GB/s (descriptor-
    serial); 4 queues reach ~126 GB/s.
  - The per-tile selection matrices M[p,j] = (j == rloc[p]) * vals[p]
    (bf16) are PRECOMPUTED ON THE HOST and streamed from HBM via HWDGE
    (one dma_start per window). Building M on-chip is a trap: DVE ops
    hold the shared SBUF port pair and starve SWDGE descriptor
    generation (gather stalls), Pool builds serialize with gather
    desc-gen, and ACT has no tensor_scalar. Streaming costs ~63MB/iter
    of spare sequential HBM bandwidth and leaves the vector engines
    idle.
  - Per window, per 128-edge tile: matmul-accumulate
    psum[j,d] += M.T @ G[:, :64] over the window's T tiles; ACT copies
    psum into a [128, W*64] SBUF accumulator; one final DMA out.
    Host re-interleaves [128, W, 64] -> [W*128, 64] and concatenates.
"""

import os

import numpy as np

N_NODES = 100000
N_EDGES = 1600000
D = 64
P = 128
NC = 8
RPC = N_NODES // NC  # rows per core = 12500
W = -(-RPC // P)  # windows per core = 98
CH = 4
CHROWS = N_NODES // CH  # 25000 rows per gather chunk (< 32768 for int16)
NQ = 4  # SWDGE queues (ucode max); per-queue gather drain is ~32 GB/s

# Stash of the last run's BassKernelResults for test.py.
LAST_RESULTS = None


def build_program(ch_rows, w, t_cs, gw, d=D, reps=1, bufs_g=3, bufs_m=4, bufs_ps=8, nq=NQ, skip_gather=False, skip_compute=False, sp_split=False):
    """Build the single-core SPMD Bass program (same on all 8 cores).

    ch_rows: rows per embeds chunk; t_cs: tiles per window per chunk.
    reps > 1 wraps the whole body in a hardware loop (benchmarking only).
    """
    import concourse.bacc as bacc
    import concourse.mybir as mybir
    import concourse.tile as tile

    f32 = mybir.dt.float32
    bf16 = mybir.dt.bfloat16
    i16 = mybir.dt.int16

    ch = len(t_cs)
    t_tot = sum(t_cs)
    c_off = np.concatenate([[0], np.cumsum([w * t for t in t_cs])])

    nc = bacc.Bacc(num_swdge_queues=nq)
    emb_ds = [
        nc.declare_dram_parameter(f"emb{c}", [ch_rows, 2 * d], bf16, isOutput=False)
        for c in range(ch)
    ]
    n_slots = w * t_tot * P
    idx_d = nc.declare_dram_parameter("idx", [P, n_slots // 16], i16, isOutput=False)
    # Host-precomputed M tiles, packed per window in compute order.
    mt_d = nc.declare_dram_parameter("mt", [P, n_slots], bf16, isOutput=False)
    out_d = nc.declare_dram_parameter("out", [P, w * d], f32, isOutput=True)

    assert w % gw == 0
    qctr = [0]
    with tile.TileContext(nc) as tc:
        with (
            tc.tile_pool(name="const", bufs=1) as cpool,
            tc.tile_pool(name="gath", bufs=bufs_g) as gpool,
            tc.tile_pool(name="m", bufs=bufs_m) as mpool,
            tc.tile_pool(name="ps", bufs=bufs_ps, space="PSUM") as ppool,
        ):
            idx_sb = cpool.tile([P, n_slots // 16], i16, name="idx_sb")
            nc.sync.dma_start(out=idx_sb[:], in_=idx_d[:])
            out_sb = cpool.tile([P, w * d], f32, name="out_sb")
            if skip_compute:
                nc.gpsimd.memset(out_sb[:], 0.0)

            def body():
                for s in range(w // gw):
                    gb3 = []
                    for c in range(ch):
                        gb = gpool.tile(
                            [P, gw * t_cs[c] * 2 * d], bf16, name=f"gb{c}", tag=f"gb{c}"
                        )
                        view = gb[:].rearrange("p (n x) -> p n x", x=2 * d)
                        gb3.append(view)
                        n_idx = gw * t_cs[c] * P
                        slot_base = (c_off[c] + s * gw * t_cs[c]) * P
                        if skip_gather:
                            continue
                        if sp_split:
                            # single_packet=True caps at 1024 indices (ring
                            # limit); split into 8-tile calls.
                            for t0 in range(0, gw * t_cs[c], 8):
                                tn = min(8, gw * t_cs[c] - t0)
                                ni = tn * P
                                sb2 = slot_base + t0 * P
                                nc.gpsimd.dma_gather(
                                    out_ap=view[:, t0 : t0 + tn, :],
                                    in_ap=emb_ds[c][:, :],
                                    idxs_ap=idx_sb[:, sb2 // 16 : (sb2 + ni) // 16],
                                    num_idxs=ni,
                                    num_idxs_reg=ni,
                                    elem_size=2 * d,
                                    single_packet=True,
                                    queue_num=qctr[0] % nq,
                                )
                                qctr[0] += 1
                        else:
                            nc.gpsimd.dma_gather(
                                out_ap=view,
                                in_ap=emb_ds[c][:, :],
                                idxs_ap=idx_sb[
                                    :, slot_base // 16 : (slot_base + n_idx) // 16
                                ],
                                num_idxs=n_idx,
                                num_idxs_reg=n_idx,
                                elem_size=2 * d,
                                single_packet=False,
                                queue_num=qctr[0] % nq,
                            )
                            qctr[0] += 1
                    for wi in range(gw):
                        if skip_compute:
                            break
                        wid = s * gw + wi
                        mtile = mpool.tile([P, t_tot * P], bf16, name="mt_sb")
                        nc.sync.dma_start(
                            out=mtile[:],
                            in_=mt_d[:, wid * t_tot * P : (wid + 1) * t_tot * P],
                        )
                        ps = ppool.tile([P, d], f32, space="PSUM", name="ps")
                        t_ctr = 0
                        for c in range(ch):
                            for j in range(t_cs[c]):
                                nc.tensor.matmul(
                                    out=ps[:],
                                    lhsT=mtile[:, t_ctr * P : (t_ctr + 1) * P],
                                    rhs=gb3[c][:, wi * t_cs[c] + j, 0:d],
                                    start=(t_ctr == 0),
                                    stop=(t_ctr == t_tot - 1),
                                )
                                t_ctr += 1
                        nc.scalar.copy(
                            out=out_sb[:, wid * d : (wid + 1) * d], in_=ps[:]
                        )
                nc.sync.dma_start(out=out_d[:], in_=out_sb[:])

            if reps == 1:
                body()
            else:
                with tc.For_i(0, reps, 1):
                    body()
    nc.compile()
    return nc


def prep_shards(rows, cols, vals):
    """Pack edges into chunk-major slot arrays.

    Returns (idx16 [NC,128,nslots/16], mt [NC,128,W*T*128] bf16, t_cs).
    Slot (gt, p): gt = C_off[c] + w*T_c[c] + j. mt holds the per-tile
    selection matrices in per-window compute order:
    mt[p, (w*T + seq)*128 + j] = (j == rloc) * val for the edge at
    (tile seq of window w, partition p).
    """
    import ml_dtypes

    rows = np.asarray(rows).astype(np.int64)
    cols = np.asarray(cols).astype(np.int64)
    vals = np.asarray(vals).astype(np.float32)
    e = rows.shape[0]

    k = rows // RPC
    lr = rows - k * RPC
    wv = lr // P
    rloc_v = lr - wv * P
    cv = cols // CHROWS
    idxloc = (cols - cv * CHROWS).astype(np.int16)

    # group edges by (k, c, w), ascending col within each run (HBM locality)
    perm = np.lexsort((idxloc, wv, cv, k))
    k_s, c_s, w_s = k[perm], cv[perm], wv[perm]
    key = (k_s * CH + c_s) * W + w_s
    counts = np.bincount(key, minlength=NC * CH * W)
    t_need = -(-counts // P).reshape(NC, CH, W)
    t_cs = [int(t_need[:, c, :].max()) for c in range(CH)]
    t_tot = sum(t_cs)
    c_off = np.concatenate([[0], np.cumsum([W * t for t in t_cs])])
    seq_off = np.concatenate([[0], np.cumsum(t_cs)])

    starts = np.concatenate([[0], np.cumsum(counts)])
    q = np.arange(e) - np.repeat(starts[:-1], counts)  # pos within (k,c,w) run
    j = q // P
    p = q % P
    tc_arr = np.array(t_cs)[c_s]
    gt = c_off[c_s] + w_s * tc_arr + j  # global tile id per edge (gather order)
    seq = seq_off[c_s] + j  # tile index within the window, compute order

    n_slots = W * t_tot * P
    idx16 = np.zeros((NC, 16, n_slots // 16), np.int16)
    mt = np.zeros((NC, P, n_slots), ml_dtypes.bfloat16)

    slot = gt * P + p  # global flat slot
    idx16[k_s, slot % 16, slot // 16] = idxloc[perm]
    mcol = (w_s * t_tot + seq) * P + rloc_v[perm]
    mt[k_s, p, mcol] = vals[perm].astype(ml_dtypes.bfloat16)
    # replicate the 16-partition index block for the 8 Q7 cores
    idx128 = np.tile(idx16, (1, 8, 1))
    return idx128, mt, t_cs


def make_emb_chunks(embeds):
    """Overlapping bf16 pair table: row i = [bf16(emb[i]) | bf16(emb[i+1])],
    split into CH contiguous chunks of CHROWS rows (256B each)."""
    import ml_dtypes

    emb = np.asarray(embeds).astype(ml_dtypes.bfloat16)
    tab = np.zeros((N_NODES, 2 * D), ml_dtypes.bfloat16)
    tab[:, :D] = emb
    tab[:-1, D:] = emb[1:]
    return [
        np.ascontiguousarray(tab[c * CHROWS : (c + 1) * CHROWS]) for c in range(CH)
    ]


def make_in_maps(rows, cols, vals, embeds):
    idx128, mt, t_cs = prep_shards(rows, cols, vals)
    emb_chunks = make_emb_chunks(embeds)
    in_maps = []
    for c in range(NC):
        m = {f"emb{i}": emb_chunks[i] for i in range(CH)}
        m["idx"] = np.ascontiguousarray(idx128[c])
        m["mt"] = np.ascontiguousarray(mt[c])
        in_maps.append(m)
    return in_maps, t_cs


def unshard_output(results):
    blocks = []
    for c in range(NC):
        o = results[c]["out"].reshape(P, W, D)
        blocks.append(o.transpose(1, 0, 2).reshape(W * P, D)[:RPC])
    return np.ascontiguousarray(np.concatenate(blocks, axis=0), dtype=np.float32)


def kernel(rows, cols, vals, embeds):
    global LAST_RESULTS
    from concourse.bass_utils import run_bass_kernel_spmd

    in_maps, t_cs = make_in_maps(rows, cols, vals, embeds)
    gw = 7 if W % 7 == 0 else 1
    nc = build_program(CHROWS, W, t_cs, gw)

    res = run_bass_kernel_spmd(
        nc,
        in_maps,
        core_ids=list(range(NC)),
        trace=bool(int(os.environ.get("GCN_TRACE", "0"))),
    )
    LAST_RESULTS = res
    return unshard_output(res.results)
